# revision 3
# baseline (speedup 1.0000x reference)
"""PositionalGAT layer on 8 Trainium2 NeuronCores (Bass/Tile) — v4.

Sharding: dst-partitioned nodes (graph parallel); node table replicated.

v4 layout: each core's m dst nodes are DEGREE-SORTED and binned into KT
bins of 128; bin k's node at rank p lives on PARTITION p. An edge tile is
one COLUMN [128, 1]: at most one edge per node. Consequences:
  - s_dst is a per-partition constant per bin (one tiny gather per bin,
    broadcast across columns) — no per-edge dst gather at all.
  - the per-bin aggregation is matmul(lhsT=IDENTITY, rhs=msg_col)
    accumulated in PSUM — no one-hot build, no rel metadata.
  - pad slots use a sentinel src index; bounds_check skips their fetch
    (no DMA traffic), and a -100 ss pre-fill makes their ex ~ 0.
Degree-sorted binning makes sum(max-degree per bin) ~ 1.03x E/128, so
the single-index gather count (the Pool/SWDGE bottleneck: one 128-row
indirect gather per column) is near minimal.

phase 1 (unchanged from v2): tbl[NP, 264] bf16 = [ft(256, (d,h) order) |
s_src(4) | s_dst(4)] via bf16 PE matmuls of featT against host-folded
weights. The (d,h) column order keeps the per-head ex broadcast off the
last AP dim so DVE runs its 2x bf16 mode.

Host does integer graph preprocessing (sort/bin/permute), layout, and
dtype casts; the only host FP math is folding attn_src/attn_dst into W
(parameter prep, as in the baseline). Outputs come back permuted and the
host inverse-permutes rows (integer indexing).
"""

import numpy as np
import ml_dtypes

import concourse.bass as bass
import concourse.mybir as mybir
import concourse.tile as tile
from concourse import bacc
from concourse.bass import IndirectOffsetOnAxis
from concourse.bass_utils import run_bass_kernel_spmd

F32 = mybir.dt.float32
BF16 = mybir.dt.bfloat16
I32 = mybir.dt.int32
BF = ml_dtypes.bfloat16

N, E, H, D, P = 50000, 800000, 4, 64, 16
IN = 256
C = IN - P               # 240
HD = H * D               # 256
TW = HD + 8              # 264: ft | s_src(4) | s_dst(4)
RW = HD + 4              # 260: msg | ex
NCORES = 8
PT = 128
KC = 16                  # edge columns per compute chunk
GRP = 16                 # node tiles per phase-1 group
NQ = 4                   # SWDGE queues to spread indirect gathers over


def _pad128(x):
    return (x + 127) // 128 * 128


# --------------------------------------------------------------------------
# host-side graph preprocessing (integer only)
# --------------------------------------------------------------------------

def prep_edges(src, dst, n_nodes, n_cores):
    """Degree-sort each core's dst nodes into KT bins of 128 (partition =
    rank in bin); per bin, node edges fill columns 0..deg-1. The per-bin
    column count KE[k] is uniform across cores (max) so one SPMD program
    serves all cores.

    Returns (KE, per_core); per_core[c] = dict(
      srcm  [sum(KE)*128] int32 : flat [bin][p][col] src ids (SENT pad)
      permg [128, KT]     int32 : global node id of (bin k, partition p)
      perm  [m]           int64 : host-side inverse mapping (row -> node)
    )
    """
    m = n_nodes // n_cores
    KT = (m + PT - 1) // PT
    mp = KT * PT
    SENT = _pad128(n_nodes)  # dedicated pad row: ft=0, ss=-100 -> ex~0
    order = np.argsort(dst, kind="stable")
    dsts = dst[order].astype(np.int64)
    srcs = src[order].astype(np.int64)
    bounds = np.searchsorted(dsts, np.arange(0, n_nodes + 1, m))

    packs = []
    KE = np.ones(KT, np.int64)
    for c in range(n_cores):
        d = dsts[bounds[c]:bounds[c + 1]] - c * m
        s = srcs[bounds[c]:bounds[c + 1]]
        deg = np.bincount(d, minlength=m)
        perm = np.argsort(-deg, kind="stable")
        degp = np.zeros(mp, np.int64)
        degp[:m] = deg[perm]
        KE = np.maximum(KE, degp.reshape(KT, PT)[:, 0])
        packs.append((d, s, deg, perm))

    TK = int(KE.sum())
    b0 = np.zeros(KT, np.int64)
    b0[1:] = np.cumsum(KE * PT)[:-1]

    per_core = []
    for c in range(n_cores):
        d, s, deg, perm = packs[c]
        slot_of = np.empty(m, np.int64)
        slot_of[perm] = np.arange(m)
        estart = np.zeros(m, np.int64)
        estart[1:] = np.cumsum(deg)[:-1]
        col = np.arange(len(d)) - estart[d]
        slot = slot_of[d]
        k = slot >> 7
        p = slot & (PT - 1)
        flat = b0[k] + p * KE[k] + col

        srcm = np.full(TK * PT, SENT, np.int64)
        srcm[flat] = s

        permp = np.empty(mp, np.int64)
        permp[:m] = c * m + perm
        permp[m:] = c * m          # pad rows: any valid id, discarded
        permg = np.ascontiguousarray(
            permp.reshape(KT, PT).T).astype(np.int32)   # [128, KT]

        per_core.append(dict(srcm=srcm.astype(np.int32), permg=permg,
                             perm=perm))
    return [int(x) for x in KE], per_core


# --------------------------------------------------------------------------
# device program
# --------------------------------------------------------------------------

def build_program(n_nodes, n_cores, KE, kc=KC, grp=GRP, nq=NQ,
                  debug_io=False):
    m = n_nodes // n_cores
    KT = (m + PT - 1) // PT
    mp = KT * PT
    NP = _pad128(n_nodes)
    NT = NP // PT
    TK = sum(KE)
    b0 = np.zeros(KT, np.int64)
    b0[1:] = np.cumsum(np.asarray(KE) * PT)[:-1]

    nc = bacc.Bacc(None, target_bir_lowering=False, debug=False,
                   num_swdge_queues=nq)
    qi = [0]

    def gather(out, in_, offset_ap, **kw):
        inst = nc.gpsimd.indirect_dma_start(
            out=out, out_offset=None, in_=in_,
            in_offset=offset_ap, **kw)
        if nq > 1:
            q = qi[0] % nq
            qi[0] += 1
            inst.ins.queue = f"qPoolDynamic{q or ''}"
        return inst

    with tile.TileContext(nc) as tc:
        with tc.tile_pool(name="dram", bufs=1, space="DRAM") as dram:
            featT = dram.tile([IN, NP], BF16, kind="ExternalInput",
                              name="featT", uniquify=False)
            wa8 = dram.tile([IN, TW], BF16, kind="ExternalInput",
                            name="wa8", uniquify=False)
            ident_in = dram.tile([PT, PT], BF16, kind="ExternalInput",
                                 name="ident", uniquify=False)
            feat_own = dram.tile([mp, IN], F32, kind="ExternalInput",
                                 name="feat_own", uniquify=False)
            srcm_t = dram.tile([TK * PT], I32, kind="ExternalInput",
                               name="srcm", uniquify=False)
            permg_t = dram.tile([PT, KT], I32, kind="ExternalInput",
                                name="permg", uniquify=False)
            out_t = dram.tile([mp, IN], F32, kind="ExternalOutput",
                              name="out", uniquify=False)
            tbl = dram.tile([NP + PT, TW], BF16, name="tbl",
                            uniquify=False, kind="Internal")
            if debug_io:
                dbg_meta = dram.tile([PT, KC], I32, name="dbg_meta",
                                     uniquify=False, kind="ExternalOutput")
                dbg_gt = dram.tile([PT, KC * RW], BF16, name="dbg_gt",
                                   uniquify=False, kind="ExternalOutput")
                dbg_sdg = dram.tile([PT, H], BF16, name="dbg_sdg",
                                    uniquify=False, kind="ExternalOutput")
                dbg_lg = dram.tile([PT, KC * H], BF16, name="dbg_lg",
                                   uniquify=False, kind="ExternalOutput")
                dbg_gt2 = dram.tile([PT, KC * RW], BF16, name="dbg_gt2",
                                    uniquify=False, kind="ExternalOutput")
                dbg_raw2 = dram.tile([PT, KC * RW], BF16, name="dbg_raw2",
                                     uniquify=False, kind="ExternalOutput")
                dbg_meta2 = dram.tile([PT, KC], I32, name="dbg_meta2",
                                      uniquify=False, kind="ExternalOutput")
                dbg_acc = dram.tile([PT, RW], F32, name="dbg_acc",
                                    uniquify=False, kind="ExternalOutput")

            # ---------------- phase 1: node table -----------------------
            with tc.tile_pool(name="const1", bufs=1) as cpool, \
                 tc.tile_pool(name="p1", bufs=2) as pool, \
                 tc.tile_pool(name="ps1", bufs=8, space="PSUM") as psp:
                wa0 = cpool.tile([PT, TW], BF16)
                wa1 = cpool.tile([PT, TW], BF16)
                nc.sync.dma_start(out=wa0[:], in_=wa8[0:PT, :])
                nc.sync.dma_start(out=wa1[:], in_=wa8[PT:IN, :])

                for g0 in range(0, NT, grp):
                    gn = min(grp, NT - g0)
                    cols = gn * PT
                    fT0 = pool.tile([PT, grp * PT], BF16, tag="fT0")
                    fT1 = pool.tile([PT, grp * PT], BF16, tag="fT1")
                    nc.sync.dma_start(
                        out=fT0[:, :cols],
                        in_=featT[0:PT, g0 * PT:g0 * PT + cols])
                    nc.sync.dma_start(
                        out=fT1[:, :cols],
                        in_=featT[PT:IN, g0 * PT:g0 * PT + cols])
                    stage = pool.tile([PT, grp * TW], BF16, tag="stage")
                    for j in range(gn):
                        ps = psp.tile([PT, TW], F32, tag="ps")
                        nc.tensor.matmul(
                            out=ps[:], lhsT=fT0[:, j * PT:(j + 1) * PT],
                            rhs=wa0[:], start=True, stop=False)
                        nc.tensor.matmul(
                            out=ps[:], lhsT=fT1[:, j * PT:(j + 1) * PT],
                            rhs=wa1[:], start=False, stop=True)
                        dstg = stage[:, j * TW:(j + 1) * TW]
                        if j % 2 == 0:
                            nc.vector.tensor_copy(out=dstg, in_=ps[:])
                        else:
                            nc.scalar.copy(out=dstg, in_=ps[:])
                    nc.sync.dma_start(
                        out=tbl[g0 * PT:g0 * PT + gn * PT, :].rearrange(
                            "(j p) w -> p j w", p=PT),
                        in_=stage[:, :gn * TW].rearrange(
                            "p (j w) -> p j w", w=TW))
                padr = pool.tile([PT, TW], BF16, tag="padr")
                nc.vector.memset(padr[:], 0.0)
                nc.vector.memset(padr[:, HD:HD + 4], -100.0)
                nc.sync.dma_start(out=tbl[NP:NP + PT, :], in_=padr[:])

            # ---------------- phase 2: edges + aggregate -----------------
            with tc.tile_pool(name="const2", bufs=1) as cpool2, \
                 tc.tile_pool(name="p2", bufs=3) as pool, \
                 tc.tile_pool(name="p3", bufs=3) as pool3, \
                 tc.tile_pool(name="ps2", bufs=4, space="PSUM") as apool:
                ident_sb = cpool2.tile([PT, PT], BF16)
                nc.sync.dma_start(out=ident_sb[:], in_=ident_in[:, :])
                permg_sb = cpool2.tile([PT, KT], I32)
                nc.sync.dma_start(out=permg_sb[:], in_=permg_t[:, :])

                nchunk = 0
                for k in range(KT):
                    ke = KE[k]
                    sdg = pool3.tile([PT, H], BF16, tag="sdg")
                    gather(sdg[:], tbl[:, :],
                           IndirectOffsetOnAxis(
                               ap=permg_sb[:, k:k + 1], axis=0),
                           element_offset=RW)
                    ftl = pool3.tile([PT, IN], F32, tag="ftl")
                    nc.sync.dma_start(
                        out=ftl[:], in_=feat_own[k * PT:(k + 1) * PT, :])
                    acc = apool.tile([PT, RW], F32, tag="acc", name="acc")

                    base = int(b0[k])
                    for c0 in range(0, ke, kc):
                        cn = min(kc, ke - c0)
                        meta = pool.tile([PT, kc], I32, tag="meta")
                        src_ap = srcm_t[base:base + PT * ke].rearrange(
                            "(p c) -> p c", c=ke)
                        nc.sync.dma_start(out=meta[:, :cn],
                                          in_=src_ap[:, c0:c0 + cn])
                        gt = pool.tile([PT, kc * RW], BF16, tag="gt")
                        gt3 = gt[:].rearrange("p (g w) -> p g w", g=kc)
                        for j in range(cn):
                            gather(gt3[:, j, :], tbl[:, :],
                                   IndirectOffsetOnAxis(
                                       ap=meta[:, j:j + 1], axis=0))

                        if debug_io and k == 0 and c0 > 0:
                            nc.sync.dma_start(out=dbg_raw2[:, :], in_=gt[:])
                            nc.sync.dma_start(out=dbg_meta2[:, :], in_=meta[:])
                        # logits -> leaky_relu -> exp (into gt ss slots)
                        lg = pool.tile([PT, kc * H], BF16, tag="lg")
                        lg3 = lg[:].rearrange("p (g w) -> p g w", g=kc)
                        nc.vector.tensor_tensor(
                            out=lg3[:, :cn], in0=gt3[:, :cn, HD:RW],
                            in1=sdg[:].unsqueeze(1).to_broadcast(
                                [PT, cn, H]),
                            op=mybir.AluOpType.add)
                        lr = pool.tile([PT, kc * H], BF16, tag="lr")
                        nc.vector.tensor_scalar_mul(
                            out=lr[:, :cn * H], in0=lg[:, :cn * H],
                            scalar1=0.2)
                        nc.vector.tensor_tensor(
                            out=lr[:, :cn * H], in0=lg[:, :cn * H],
                            in1=lr[:, :cn * H], op=mybir.AluOpType.max)
                        nc.scalar.activation(
                            out=gt3[:, :cn, HD:RW],
                            in_=lr[:].rearrange(
                                "p (g w) -> p g w", g=kc)[:, :cn],
                            func=mybir.ActivationFunctionType.Exp)

                        # msg = ft * ex in place ((d,h) order: ex broadcast
                        # on a middle dim keeps DVE 2x bf16 mode)
                        exb = gt3[:, :cn, HD:RW].unsqueeze(2).to_broadcast(
                            [PT, cn, D, H])
                        ftv = gt3[:, :cn, 0:HD].rearrange(
                            "p g (d h) -> p g d h", h=H)
                        nc.vector.tensor_tensor(
                            out=ftv, in0=ftv, in1=exb,
                            op=mybir.AluOpType.mult)

                        if debug_io and k == 0 and c0 == 0:
                            nc.sync.dma_start(out=dbg_meta[:, :], in_=meta[:])
                            nc.sync.dma_start(out=dbg_gt[:, :], in_=gt[:])
                            nc.sync.dma_start(out=dbg_sdg[:, :], in_=sdg[:])
                            nc.sync.dma_start(out=dbg_lg[:, :], in_=lg[:])
                        if debug_io and k == 0 and c0 > 0:
                            nc.sync.dma_start(out=dbg_gt2[:, :], in_=gt[:])
                        for j in range(cn):
                            nc.tensor.matmul(
                                out=acc[:], lhsT=ident_sb[:],
                                rhs=gt3[:, j, :],
                                start=(c0 == 0 and j == 0),
                                stop=(c0 + j == ke - 1))
                        nchunk += 1

                    if debug_io and k == 0:
                        acst = pool3.tile([PT, RW], F32, tag="acst")
                        nc.vector.tensor_copy(out=acst[:], in_=acc[:])
                        nc.sync.dma_start(out=dbg_acc[:, :], in_=acst[:])
                    # normalize + residual + store (rows stay permuted)
                    dn = pool3.tile([PT, H], F32, tag="dn")
                    nc.vector.tensor_scalar_max(
                        out=dn[:], in0=acc[:, HD:RW], scalar1=1e-30)
                    rc = pool3.tile([PT, H], F32, tag="rc")
                    nc.vector.reciprocal(rc[:], dn[:])
                    ot = pool3.tile([PT, IN], F32, tag="ot")
                    nc.vector.tensor_tensor(
                        out=ot[:].rearrange("p (h d) -> p h d", d=D),
                        in0=acc[:, 0:HD].rearrange("p (d h) -> p h d", h=H),
                        in1=rc[:].to_broadcast([PT, H, D]),
                        op=mybir.AluOpType.mult)
                    nc.vector.tensor_tensor(
                        out=ot[:], in0=ot[:], in1=ftl[:],
                        op=mybir.AluOpType.add)
                    nc.sync.dma_start(
                        out=out_t[k * PT:(k + 1) * PT, :], in_=ot[:])

    nc.compile()
    return nc


# --------------------------------------------------------------------------
# host wrapper
# --------------------------------------------------------------------------

def prep_inputs(feat, src, dst, W, attn_src, attn_dst, pos_attn_src,
                pos_attn_dst, n_nodes, n_cores):
    m = n_nodes // n_cores
    KT = (m + PT - 1) // PT
    mp = KT * PT
    NP = _pad128(n_nodes)

    featp = np.zeros((NP, IN), np.float32)
    featp[:n_nodes] = feat
    featT = np.ascontiguousarray(featp.T).astype(BF)

    wa8 = np.zeros((IN, TW), np.float32)
    wa8[:C, :HD] = W
    wr = W.reshape(C, H, D)
    wa8[:C, HD:HD + 4] = np.einsum("chd,hd->ch", wr, attn_src[0])
    wa8[:C, HD + 4:] = np.einsum("chd,hd->ch", wr, attn_dst[0])
    wa8[C:, HD:HD + 4] = pos_attn_src[0].T
    wa8[C:, HD + 4:] = pos_attn_dst[0].T
    # ft columns in (d, h) order (see build_program docstring)
    wa8[:, :HD] = np.ascontiguousarray(
        wa8[:, :HD].reshape(IN, H, D).transpose(0, 2, 1).reshape(IN, HD))
    wa8 = wa8.astype(BF)

    ident = np.eye(PT, dtype=np.float32).astype(BF)

    KE, per_core = prep_edges(src, dst, n_nodes, n_cores)

    in_maps = []
    for c in range(n_cores):
        pc = per_core[c]
        permp = np.empty(mp, np.int64)
        permp[:m] = pc["perm"]
        permp[m:] = 0
        fo = np.ascontiguousarray(
            feat[c * m + permp], dtype=np.float32)
        in_maps.append(dict(
            featT=featT, wa8=wa8, ident=ident, feat_own=fo,
            srcm=pc["srcm"], permg=pc["permg"],
        ))
    return KE, in_maps, [pc["perm"] for pc in per_core]


_PROG_CACHE = {}


def _get_program(n_nodes, n_cores, KE):
    key = (n_nodes, n_cores, tuple(KE), KC, NQ)
    if key not in _PROG_CACHE:
        _PROG_CACHE[key] = build_program(n_nodes, n_cores, KE)
    return _PROG_CACHE[key]


def run(feat, src, dst, W, attn_src, attn_dst, pos_attn_src, pos_attn_dst,
        n_nodes=N, n_cores=NCORES, trace=False):
    m = n_nodes // n_cores
    KE, in_maps, perms = prep_inputs(
        feat, src, dst, W, attn_src, attn_dst, pos_attn_src, pos_attn_dst,
        n_nodes, n_cores)
    nc = _get_program(n_nodes, n_cores, KE)
    res = run_bass_kernel_spmd(nc, in_maps, core_ids=list(range(n_cores)),
                               trace=trace)
    out = np.empty((n_nodes, IN), np.float32)
    for c in range(n_cores):
        rows = res.results[c]["out"]
        out[c * m + perms[c]] = rows[:m]
    return out, res


def make_bench(nc, in_maps, n_cores):
    """Steady-state exec timer: jitted shard_map, device-resident inputs."""
    import jax
    from jax.sharding import Mesh, PartitionSpec
    from jax.experimental.shard_map import shard_map
    import concourse.mybir as mybir_
    from concourse import bass2jax as b2j

    b2j.install_neuronx_cc_hook()
    fn = nc.m.functions[0]
    partition_name = (nc.partition_id_tensor.name
                      if nc.partition_id_tensor else None)
    in_names, out_names, out_avals, zero_outs = [], [], [], []
    for alloc in fn.allocations:
        if not isinstance(alloc, mybir_.MemoryLocationSet):
            continue
        name = alloc.memorylocations[0].name
        if alloc.kind == "ExternalInput":
            if name != partition_name:
                in_names.append(name)
        elif alloc.kind == "ExternalOutput":
            shape = tuple(alloc.tensor_shape)
            dtype = mybir_.dt.np(alloc.dtype)
            out_names.append(name)
            out_avals.append(jax.core.ShapedArray(shape, dtype))
            zero_outs.append(np.zeros(shape, dtype))
    n_params = len(in_names)
    all_names = in_names + out_names
    if partition_name is not None:
        all_names = all_names + [partition_name]

    def _body(*args):
        operands = list(args)
        if partition_name is not None:
            operands.append(b2j.partition_id_tensor())
        outs = b2j._bass_exec_p.bind(
            *operands, out_avals=tuple(out_avals), in_names=tuple(all_names),
            out_names=tuple(out_names), lowering_input_output_aliases=(),
            sim_require_finite=False, sim_require_nnan=False, nc=nc)
        return tuple(outs)

    devices = jax.devices()[:n_cores]
    mesh = Mesh(np.asarray(devices), ("core",))
    nio = n_params + len(out_names)
    sharded = jax.jit(shard_map(
        _body, mesh=mesh, in_specs=(PartitionSpec("core"),) * nio,
        out_specs=(PartitionSpec("core"),) * len(out_names),
        check_rep=False), keep_unused=True)
    concat_in = [
        np.concatenate([np.asarray(in_maps[c][nm]) for c in range(n_cores)],
                       axis=0) for nm in in_names]
    concat_zo = [np.concatenate([z] * n_cores, axis=0) for z in zero_outs]
    sharding = jax.sharding.NamedSharding(mesh, PartitionSpec("core"))
    dev_in = [jax.device_put(a, sharding) for a in concat_in]
    dev_zo = [jax.device_put(a, sharding) for a in concat_zo]

    def run_once():
        outs = sharded(*dev_in, *dev_zo)
        jax.block_until_ready(outs)
        return {nm: np.asarray(o) for nm, o in zip(out_names, outs)}

    def time_runs(k=10):
        import time as _t
        ts = []
        for _ in range(k):
            t0 = _t.perf_counter()
            jax.block_until_ready(sharded(*dev_in, *dev_zo))
            ts.append(_t.perf_counter() - t0)
        return ts

    return run_once, time_runs


def kernel(feat, src, dst, W, attn_src, attn_dst, pos_attn_src,
           pos_attn_dst):
    out, _ = run(np.asarray(feat, np.float32), np.asarray(src),
                 np.asarray(dst), np.asarray(W, np.float32),
                 np.asarray(attn_src, np.float32),
                 np.asarray(attn_dst, np.float32),
                 np.asarray(pos_attn_src, np.float32),
                 np.asarray(pos_attn_dst, np.float32))
    return out


# revision 5
# speedup vs baseline: 1.1077x; 1.1077x over previous
"""PositionalGAT layer on 8 Trainium2 NeuronCores (Bass/Tile) — v4.

Sharding: dst-partitioned nodes (graph parallel); node table replicated.

v4 layout: each core's m dst nodes are DEGREE-SORTED and binned into KT
bins of 128; bin k's node at rank p lives on PARTITION p. An edge tile is
one COLUMN [128, 1]: at most one edge per node. Consequences:
  - s_dst is a per-partition constant per bin (one tiny gather per bin,
    broadcast across columns) — no per-edge dst gather at all.
  - the per-bin aggregation is matmul(lhsT=IDENTITY, rhs=msg_col)
    accumulated in PSUM — no one-hot build, no rel metadata.
  - pad slots use a sentinel src index; bounds_check skips their fetch
    (no DMA traffic), and a -100 ss pre-fill makes their ex ~ 0.
Degree-sorted binning makes sum(max-degree per bin) ~ 1.03x E/128, so
the single-index gather count (the Pool/SWDGE bottleneck: one 128-row
indirect gather per column) is near minimal.

phase 1 (unchanged from v2): tbl[NP, 264] bf16 = [ft(256, (d,h) order) |
s_src(4) | s_dst(4)] via bf16 PE matmuls of featT against host-folded
weights. The (d,h) column order keeps the per-head ex broadcast off the
last AP dim so DVE runs its 2x bf16 mode.

Host does integer graph preprocessing (sort/bin/permute), layout, and
dtype casts; the only host FP math is folding attn_src/attn_dst into W
(parameter prep, as in the baseline). Outputs come back permuted and the
host inverse-permutes rows (integer indexing).
"""

import numpy as np
import ml_dtypes

import concourse.bass as bass
import concourse.mybir as mybir
import concourse.tile as tile
from concourse import bacc
from concourse.bass import IndirectOffsetOnAxis
from concourse.bass_utils import run_bass_kernel_spmd

F32 = mybir.dt.float32
BF16 = mybir.dt.bfloat16
I32 = mybir.dt.int32
BF = ml_dtypes.bfloat16

N, E, H, D, P = 50000, 800000, 4, 64, 16
IN = 256
C = IN - P               # 240
HD = H * D               # 256
TW = HD + 8              # 264: ft | s_src(4) | s_dst(4)
RW = HD + 4              # 260: msg | ex
NCORES = 8
PT = 128
KC = 32                  # edge columns per compute chunk
GRP = 16                 # node tiles per phase-1 group
NQ = 4                   # SWDGE queues to spread indirect gathers over


def _pad128(x):
    return (x + 127) // 128 * 128


# --------------------------------------------------------------------------
# host-side graph preprocessing (integer only)
# --------------------------------------------------------------------------

def prep_edges(src, dst, n_nodes, n_cores):
    """Degree-sort each core's dst nodes into KT bins of 128 (partition =
    rank in bin); per bin, node edges fill columns 0..deg-1. The per-bin
    column count KE[k] is uniform across cores (max) so one SPMD program
    serves all cores.

    Returns (KE, per_core); per_core[c] = dict(
      srcm  [sum(KE)*128] int32 : flat [bin][p][col] src ids (SENT pad)
      permg [128, KT]     int32 : global node id of (bin k, partition p)
      perm  [m]           int64 : host-side inverse mapping (row -> node)
    )
    """
    m = n_nodes // n_cores
    KT = (m + PT - 1) // PT
    mp = KT * PT
    SENT = _pad128(n_nodes)  # dedicated pad row: ft=0, ss=-100 -> ex~0
    order = np.argsort(dst, kind="stable")
    dsts = dst[order].astype(np.int64)
    srcs = src[order].astype(np.int64)
    bounds = np.searchsorted(dsts, np.arange(0, n_nodes + 1, m))

    packs = []
    KE = np.ones(KT, np.int64)
    for c in range(n_cores):
        d = dsts[bounds[c]:bounds[c + 1]] - c * m
        s = srcs[bounds[c]:bounds[c + 1]]
        deg = np.bincount(d, minlength=m)
        perm = np.argsort(-deg, kind="stable")
        degp = np.zeros(mp, np.int64)
        degp[:m] = deg[perm]
        KE = np.maximum(KE, degp.reshape(KT, PT)[:, 0])
        packs.append((d, s, deg, perm))

    TK = int(KE.sum())
    b0 = np.zeros(KT, np.int64)
    b0[1:] = np.cumsum(KE * PT)[:-1]

    per_core = []
    for c in range(n_cores):
        d, s, deg, perm = packs[c]
        slot_of = np.empty(m, np.int64)
        slot_of[perm] = np.arange(m)
        estart = np.zeros(m, np.int64)
        estart[1:] = np.cumsum(deg)[:-1]
        col = np.arange(len(d)) - estart[d]
        slot = slot_of[d]
        k = slot >> 7
        p = slot & (PT - 1)
        flat = b0[k] + p * KE[k] + col

        srcm = np.full(TK * PT, SENT, np.int64)
        srcm[flat] = s

        permp = np.empty(mp, np.int64)
        permp[:m] = c * m + perm
        permp[m:] = c * m          # pad rows: any valid id, discarded
        permg = np.ascontiguousarray(
            permp.reshape(KT, PT).T).astype(np.int32)   # [128, KT]

        per_core.append(dict(srcm=srcm.astype(np.int32), permg=permg,
                             perm=perm))
    return [int(x) for x in KE], per_core


# --------------------------------------------------------------------------
# device program
# --------------------------------------------------------------------------

def build_program(n_nodes, n_cores, KE, kc=KC, grp=GRP, nq=NQ,
                  debug_io=False):
    m = n_nodes // n_cores
    KT = (m + PT - 1) // PT
    mp = KT * PT
    NP = _pad128(n_nodes)
    NT = NP // PT
    TK = sum(KE)
    b0 = np.zeros(KT, np.int64)
    b0[1:] = np.cumsum(np.asarray(KE) * PT)[:-1]

    nc = bacc.Bacc(None, target_bir_lowering=False, debug=False,
                   num_swdge_queues=nq)
    qi = [0]

    def gather(out, in_, offset_ap, **kw):
        inst = nc.gpsimd.indirect_dma_start(
            out=out, out_offset=None, in_=in_,
            in_offset=offset_ap, **kw)
        if nq > 1:
            q = qi[0] % nq
            qi[0] += 1
            inst.ins.queue = f"qPoolDynamic{q or ''}"
        return inst

    with tile.TileContext(nc) as tc:
        with tc.tile_pool(name="dram", bufs=1, space="DRAM") as dram:
            featT = dram.tile([IN, NP], BF16, kind="ExternalInput",
                              name="featT", uniquify=False)
            wa8 = dram.tile([IN, TW], BF16, kind="ExternalInput",
                            name="wa8", uniquify=False)
            ident_in = dram.tile([PT, PT], BF16, kind="ExternalInput",
                                 name="ident", uniquify=False)
            feat_own = dram.tile([mp, IN], F32, kind="ExternalInput",
                                 name="feat_own", uniquify=False)
            srcm_t = dram.tile([TK * PT], I32, kind="ExternalInput",
                               name="srcm", uniquify=False)
            permg_t = dram.tile([PT, KT], I32, kind="ExternalInput",
                                name="permg", uniquify=False)
            out_t = dram.tile([mp, IN], F32, kind="ExternalOutput",
                              name="out", uniquify=False)
            tbl = dram.tile([NP + PT, TW], BF16, name="tbl",
                            uniquify=False, kind="Internal")
            if debug_io:
                dbg_meta = dram.tile([PT, KC], I32, name="dbg_meta",
                                     uniquify=False, kind="ExternalOutput")
                dbg_gt = dram.tile([PT, KC * RW], BF16, name="dbg_gt",
                                   uniquify=False, kind="ExternalOutput")
                dbg_sdg = dram.tile([PT, H], BF16, name="dbg_sdg",
                                    uniquify=False, kind="ExternalOutput")
                dbg_lg = dram.tile([PT, KC * H], BF16, name="dbg_lg",
                                   uniquify=False, kind="ExternalOutput")
                dbg_gt2 = dram.tile([PT, KC * RW], BF16, name="dbg_gt2",
                                    uniquify=False, kind="ExternalOutput")
                dbg_raw2 = dram.tile([PT, KC * RW], BF16, name="dbg_raw2",
                                     uniquify=False, kind="ExternalOutput")
                dbg_meta2 = dram.tile([PT, KC], I32, name="dbg_meta2",
                                      uniquify=False, kind="ExternalOutput")
                dbg_acc = dram.tile([PT, RW], F32, name="dbg_acc",
                                    uniquify=False, kind="ExternalOutput")

            # phase-2 constants: no tbl dependency, prefetch during ph1
            cpool2_cm = tc.tile_pool(name="const2", bufs=1)
            cpool2 = cpool2_cm.__enter__()
            ident_sb = cpool2.tile([PT, PT], BF16)
            nc.sync.dma_start(out=ident_sb[:], in_=ident_in[:, :])
            permg_sb = cpool2.tile([PT, KT], I32)
            nc.sync.dma_start(out=permg_sb[:], in_=permg_t[:, :])

            # ---------------- phase 1: node table -----------------------
            with tc.tile_pool(name="const1", bufs=1) as cpool, \
                 tc.tile_pool(name="p1", bufs=2) as pool, \
                 tc.tile_pool(name="ps1", bufs=8, space="PSUM") as psp:
                wa0 = cpool.tile([PT, TW], BF16)
                wa1 = cpool.tile([PT, TW], BF16)
                nc.sync.dma_start(out=wa0[:], in_=wa8[0:PT, :])
                nc.sync.dma_start(out=wa1[:], in_=wa8[PT:IN, :])

                for g0 in range(0, NT, grp):
                    gn = min(grp, NT - g0)
                    cols = gn * PT
                    fT0 = pool.tile([PT, grp * PT], BF16, tag="fT0")
                    fT1 = pool.tile([PT, grp * PT], BF16, tag="fT1")
                    nc.sync.dma_start(
                        out=fT0[:, :cols],
                        in_=featT[0:PT, g0 * PT:g0 * PT + cols])
                    nc.sync.dma_start(
                        out=fT1[:, :cols],
                        in_=featT[PT:IN, g0 * PT:g0 * PT + cols])
                    stage = pool.tile([PT, grp * TW], BF16, tag="stage")
                    for j in range(gn):
                        ps = psp.tile([PT, TW], F32, tag="ps")
                        nc.tensor.matmul(
                            out=ps[:], lhsT=fT0[:, j * PT:(j + 1) * PT],
                            rhs=wa0[:], start=True, stop=False)
                        nc.tensor.matmul(
                            out=ps[:], lhsT=fT1[:, j * PT:(j + 1) * PT],
                            rhs=wa1[:], start=False, stop=True)
                        dstg = stage[:, j * TW:(j + 1) * TW]
                        if j % 2 == 0:
                            nc.vector.tensor_copy(out=dstg, in_=ps[:])
                        else:
                            nc.scalar.copy(out=dstg, in_=ps[:])
                    nc.sync.dma_start(
                        out=tbl[g0 * PT:g0 * PT + gn * PT, :].rearrange(
                            "(j p) w -> p j w", p=PT),
                        in_=stage[:, :gn * TW].rearrange(
                            "p (j w) -> p j w", w=TW))
                padr = pool.tile([PT, TW], BF16, tag="padr")
                nc.vector.memset(padr[:], 0.0)
                nc.vector.memset(padr[:, HD:HD + 4], -100.0)
                nc.sync.dma_start(out=tbl[NP:NP + PT, :], in_=padr[:])

            # ---------------- phase 2: edges + aggregate -----------------
            with tc.tile_pool(name="p2", bufs=3) as pool, \
                 tc.tile_pool(name="p3", bufs=3) as pool3, \
                 tc.tile_pool(name="ps2", bufs=4, space="PSUM") as apool:
                nchunk = 0
                for k in range(KT):
                    ke = KE[k]
                    sdg = pool3.tile([PT, H], BF16, tag="sdg")
                    gather(sdg[:], tbl[:, :],
                           IndirectOffsetOnAxis(
                               ap=permg_sb[:, k:k + 1], axis=0),
                           element_offset=RW)
                    ftl = pool3.tile([PT, IN], F32, tag="ftl")
                    nc.sync.dma_start(
                        out=ftl[:], in_=feat_own[k * PT:(k + 1) * PT, :])
                    acc = apool.tile([PT, RW], F32, tag="acc", name="acc")

                    base = int(b0[k])
                    for c0 in range(0, ke, kc):
                        cn = min(kc, ke - c0)
                        meta = pool.tile([PT, kc], I32, tag="meta")
                        src_ap = srcm_t[base:base + PT * ke].rearrange(
                            "(p c) -> p c", c=ke)
                        nc.sync.dma_start(out=meta[:, :cn],
                                          in_=src_ap[:, c0:c0 + cn])
                        gt = pool.tile([PT, kc * RW], BF16, tag="gt")
                        gt3 = gt[:].rearrange("p (g w) -> p g w", g=kc)
                        for j in range(cn):
                            gather(gt3[:, j, :], tbl[:, :],
                                   IndirectOffsetOnAxis(
                                       ap=meta[:, j:j + 1], axis=0))

                        if debug_io and k == 0 and c0 > 0:
                            nc.sync.dma_start(out=dbg_raw2[:, :], in_=gt[:])
                            nc.sync.dma_start(out=dbg_meta2[:, :], in_=meta[:])
                        # logits -> leaky_relu -> exp (into gt ss slots)
                        lg = pool.tile([PT, kc * H], BF16, tag="lg")
                        lg3 = lg[:].rearrange("p (g w) -> p g w", g=kc)
                        nc.vector.tensor_tensor(
                            out=lg3[:, :cn], in0=gt3[:, :cn, HD:RW],
                            in1=sdg[:].unsqueeze(1).to_broadcast(
                                [PT, cn, H]),
                            op=mybir.AluOpType.add)
                        lr = pool.tile([PT, kc * H], BF16, tag="lr")
                        nc.vector.tensor_scalar_mul(
                            out=lr[:, :cn * H], in0=lg[:, :cn * H],
                            scalar1=0.2)
                        nc.vector.tensor_tensor(
                            out=lr[:, :cn * H], in0=lg[:, :cn * H],
                            in1=lr[:, :cn * H], op=mybir.AluOpType.max)
                        nc.scalar.activation(
                            out=gt3[:, :cn, HD:RW],
                            in_=lr[:].rearrange(
                                "p (g w) -> p g w", g=kc)[:, :cn],
                            func=mybir.ActivationFunctionType.Exp)

                        # msg = ft * ex in place ((d,h) order: ex broadcast
                        # on a middle dim keeps DVE 2x bf16 mode)
                        exb = gt3[:, :cn, HD:RW].unsqueeze(2).to_broadcast(
                            [PT, cn, D, H])
                        ftv = gt3[:, :cn, 0:HD].rearrange(
                            "p g (d h) -> p g d h", h=H)
                        nc.vector.tensor_tensor(
                            out=ftv, in0=ftv, in1=exb,
                            op=mybir.AluOpType.mult)

                        if debug_io and k == 0 and c0 == 0:
                            nc.sync.dma_start(out=dbg_meta[:, :], in_=meta[:])
                            nc.sync.dma_start(out=dbg_gt[:, :], in_=gt[:])
                            nc.sync.dma_start(out=dbg_sdg[:, :], in_=sdg[:])
                            nc.sync.dma_start(out=dbg_lg[:, :], in_=lg[:])
                        if debug_io and k == 0 and c0 > 0:
                            nc.sync.dma_start(out=dbg_gt2[:, :], in_=gt[:])
                        for j in range(cn):
                            nc.tensor.matmul(
                                out=acc[:], lhsT=ident_sb[:],
                                rhs=gt3[:, j, :],
                                start=(c0 == 0 and j == 0),
                                stop=(c0 + j == ke - 1))
                        nchunk += 1

                    if debug_io and k == 0:
                        acst = pool3.tile([PT, RW], F32, tag="acst")
                        nc.vector.tensor_copy(out=acst[:], in_=acc[:])
                        nc.sync.dma_start(out=dbg_acc[:, :], in_=acst[:])
                    # normalize + residual + store (rows stay permuted)
                    dn = pool3.tile([PT, H], F32, tag="dn")
                    nc.vector.tensor_scalar_max(
                        out=dn[:], in0=acc[:, HD:RW], scalar1=1e-30)
                    rc = pool3.tile([PT, H], F32, tag="rc")
                    nc.vector.reciprocal(rc[:], dn[:])
                    ot = pool3.tile([PT, IN], F32, tag="ot")
                    nc.vector.tensor_tensor(
                        out=ot[:].rearrange("p (h d) -> p h d", d=D),
                        in0=acc[:, 0:HD].rearrange("p (d h) -> p h d", h=H),
                        in1=rc[:].to_broadcast([PT, H, D]),
                        op=mybir.AluOpType.mult)
                    nc.vector.tensor_tensor(
                        out=ot[:], in0=ot[:], in1=ftl[:],
                        op=mybir.AluOpType.add)
                    nc.sync.dma_start(
                        out=out_t[k * PT:(k + 1) * PT, :], in_=ot[:])
            cpool2_cm.__exit__(None, None, None)

    nc.compile()
    return nc


# --------------------------------------------------------------------------
# host wrapper
# --------------------------------------------------------------------------

def prep_inputs(feat, src, dst, W, attn_src, attn_dst, pos_attn_src,
                pos_attn_dst, n_nodes, n_cores):
    m = n_nodes // n_cores
    KT = (m + PT - 1) // PT
    mp = KT * PT
    NP = _pad128(n_nodes)

    featp = np.zeros((NP, IN), np.float32)
    featp[:n_nodes] = feat
    featT = np.ascontiguousarray(featp.T).astype(BF)

    wa8 = np.zeros((IN, TW), np.float32)
    wa8[:C, :HD] = W
    wr = W.reshape(C, H, D)
    wa8[:C, HD:HD + 4] = np.einsum("chd,hd->ch", wr, attn_src[0])
    wa8[:C, HD + 4:] = np.einsum("chd,hd->ch", wr, attn_dst[0])
    wa8[C:, HD:HD + 4] = pos_attn_src[0].T
    wa8[C:, HD + 4:] = pos_attn_dst[0].T
    # ft columns in (d, h) order (see build_program docstring)
    wa8[:, :HD] = np.ascontiguousarray(
        wa8[:, :HD].reshape(IN, H, D).transpose(0, 2, 1).reshape(IN, HD))
    wa8 = wa8.astype(BF)

    ident = np.eye(PT, dtype=np.float32).astype(BF)

    KE, per_core = prep_edges(src, dst, n_nodes, n_cores)

    in_maps = []
    for c in range(n_cores):
        pc = per_core[c]
        permp = np.empty(mp, np.int64)
        permp[:m] = pc["perm"]
        permp[m:] = 0
        fo = np.ascontiguousarray(
            feat[c * m + permp], dtype=np.float32)
        in_maps.append(dict(
            featT=featT, wa8=wa8, ident=ident, feat_own=fo,
            srcm=pc["srcm"], permg=pc["permg"],
        ))
    return KE, in_maps, [pc["perm"] for pc in per_core]


_PROG_CACHE = {}


def _get_program(n_nodes, n_cores, KE):
    key = (n_nodes, n_cores, tuple(KE), KC, NQ)
    if key not in _PROG_CACHE:
        _PROG_CACHE[key] = build_program(n_nodes, n_cores, KE)
    return _PROG_CACHE[key]


def run(feat, src, dst, W, attn_src, attn_dst, pos_attn_src, pos_attn_dst,
        n_nodes=N, n_cores=NCORES, trace=False):
    m = n_nodes // n_cores
    KE, in_maps, perms = prep_inputs(
        feat, src, dst, W, attn_src, attn_dst, pos_attn_src, pos_attn_dst,
        n_nodes, n_cores)
    nc = _get_program(n_nodes, n_cores, KE)
    res = run_bass_kernel_spmd(nc, in_maps, core_ids=list(range(n_cores)),
                               trace=trace)
    out = np.empty((n_nodes, IN), np.float32)
    for c in range(n_cores):
        rows = res.results[c]["out"]
        out[c * m + perms[c]] = rows[:m]
    return out, res


def make_bench(nc, in_maps, n_cores):
    """Steady-state exec timer: jitted shard_map, device-resident inputs."""
    import jax
    from jax.sharding import Mesh, PartitionSpec
    from jax.experimental.shard_map import shard_map
    import concourse.mybir as mybir_
    from concourse import bass2jax as b2j

    b2j.install_neuronx_cc_hook()
    fn = nc.m.functions[0]
    partition_name = (nc.partition_id_tensor.name
                      if nc.partition_id_tensor else None)
    in_names, out_names, out_avals, zero_outs = [], [], [], []
    for alloc in fn.allocations:
        if not isinstance(alloc, mybir_.MemoryLocationSet):
            continue
        name = alloc.memorylocations[0].name
        if alloc.kind == "ExternalInput":
            if name != partition_name:
                in_names.append(name)
        elif alloc.kind == "ExternalOutput":
            shape = tuple(alloc.tensor_shape)
            dtype = mybir_.dt.np(alloc.dtype)
            out_names.append(name)
            out_avals.append(jax.core.ShapedArray(shape, dtype))
            zero_outs.append(np.zeros(shape, dtype))
    n_params = len(in_names)
    all_names = in_names + out_names
    if partition_name is not None:
        all_names = all_names + [partition_name]

    def _body(*args):
        operands = list(args)
        if partition_name is not None:
            operands.append(b2j.partition_id_tensor())
        outs = b2j._bass_exec_p.bind(
            *operands, out_avals=tuple(out_avals), in_names=tuple(all_names),
            out_names=tuple(out_names), lowering_input_output_aliases=(),
            sim_require_finite=False, sim_require_nnan=False, nc=nc)
        return tuple(outs)

    devices = jax.devices()[:n_cores]
    mesh = Mesh(np.asarray(devices), ("core",))
    nio = n_params + len(out_names)
    sharded = jax.jit(shard_map(
        _body, mesh=mesh, in_specs=(PartitionSpec("core"),) * nio,
        out_specs=(PartitionSpec("core"),) * len(out_names),
        check_rep=False), keep_unused=True)
    concat_in = [
        np.concatenate([np.asarray(in_maps[c][nm]) for c in range(n_cores)],
                       axis=0) for nm in in_names]
    concat_zo = [np.concatenate([z] * n_cores, axis=0) for z in zero_outs]
    sharding = jax.sharding.NamedSharding(mesh, PartitionSpec("core"))
    dev_in = [jax.device_put(a, sharding) for a in concat_in]
    dev_zo = [jax.device_put(a, sharding) for a in concat_zo]

    def run_once():
        outs = sharded(*dev_in, *dev_zo)
        jax.block_until_ready(outs)
        return {nm: np.asarray(o) for nm, o in zip(out_names, outs)}

    def time_runs(k=10):
        import time as _t
        ts = []
        for _ in range(k):
            t0 = _t.perf_counter()
            jax.block_until_ready(sharded(*dev_in, *dev_zo))
            ts.append(_t.perf_counter() - t0)
        return ts

    return run_once, time_runs


def kernel(feat, src, dst, W, attn_src, attn_dst, pos_attn_src,
           pos_attn_dst):
    out, _ = run(np.asarray(feat, np.float32), np.asarray(src),
                 np.asarray(dst), np.asarray(W, np.float32),
                 np.asarray(attn_src, np.float32),
                 np.asarray(attn_dst, np.float32),
                 np.asarray(pos_attn_src, np.float32),
                 np.asarray(pos_attn_dst, np.float32))
    return out
